# revision 1
# baseline (speedup 1.0000x reference)
"""Multi-head self-attention Trainium2 kernel (8 NeuronCores).

Problem: x[4, 2048, 1024], H=16 heads, D=64. Sharding: core c handles
batch b = c // 2 and head-group hg = c % 2 (8 heads = 512 features).

Per-core math (F = 512 core-local features, T = 2048 tokens, C = 1024):
  QT = (Wq_s.T @ x_b.T) + bq_s          [F, T]   (feature-major)
  KT = same with Wk_s                    [F, T]
  V  = x_b @ Wv_s + bv_s                 [T, F]   (token-major)
  per head h (64-feature slice):
    scT[j, i] = KT_h.T-slices @ QT_h     (lhsT=KT tile, rhs=QT chunk)
    expT = exp(scT / 8)                  (no max subtraction: |s/8| < ~3)
    pv[0:64, i] = sum_j V65_h[j].T @ expT[j, i]   (V65 = [V_h | ones])
    pv[64, i]   = softmax denominator
    attnT_h[:, i] = pv[0:64, i] * (1 / pv[64, i])  (replicated via PE)
  o_part = attnT.T @ Wo_s                [T, C]
Host: out[b] = o_part[2b] + o_part[2b+1] + bo.
"""

import os
import sys

import numpy as np

if "/opt/trn_rl_repo" not in sys.path:
    sys.path.insert(0, "/opt/trn_rl_repo")

import concourse.bass as bass
import concourse.mybir as mybir
import concourse.tile as tile
from concourse import bacc

F32 = mybir.dt.float32
F32R = mybir.dt.float32r
AF = mybir.ActivationFunctionType

# Full-problem constants
B, N, C, H, D = 4, 2048, 1024, 16, 64
NCORES = 8
NH = 8          # heads per core
F = NH * D      # 512 core-local features
SCALE = 1.0 / 8.0  # 1/sqrt(D)


import ml_dtypes

BF16 = mybir.dt.bfloat16


def build_attention_kernel(tok=N, cin=C, nh=NH, mm_dt=F32R, debug=False):
    """Build the per-core Bass program. Returns the finalized Bass object.

    tok: sequence length, cin: model dim (= Wo output dim), nh: heads/core.
    """
    f = nh * D
    assert tok % 512 == 0 and cin % 128 == 0 and f % 128 == 0
    c_t = cin // 128       # contraction tiles for projections
    f_t = f // 128         # feature tiles (Q/K partition tiles)
    t_t = tok // 128       # token tiles
    iw = min(1024, tok)    # exp width (psum banks spanned = iw/512)
    n_ic = tok // iw
    nsub = iw // 512
    ocw = min(512, cin)    # output-proj column chunk width
    n_oc = cin // ocw

    nc = bacc.Bacc("TRN2", target_bir_lowering=False, debug=False,
                   num_devices=NCORES)

    xT = nc.dram_tensor("xT", [cin, tok], mm_dt, kind="ExternalInput").ap()
    wq = nc.dram_tensor("wq", [cin, f], mm_dt, kind="ExternalInput").ap()
    wk = nc.dram_tensor("wk", [cin, f], mm_dt, kind="ExternalInput").ap()
    wv = nc.dram_tensor("wv", [cin, f], mm_dt, kind="ExternalInput").ap()
    bq = nc.dram_tensor("bq", [f, 1], F32, kind="ExternalInput").ap()
    bk = nc.dram_tensor("bk", [f, 1], F32, kind="ExternalInput").ap()
    bv = nc.dram_tensor("bv", [1, f], mm_dt, kind="ExternalInput").ap()
    wo = nc.dram_tensor("wo", [f, cin], mm_dt, kind="ExternalInput").ap()
    o_part = nc.dram_tensor("o_part", [tok, cin], F32,
                            kind="ExternalOutput").ap()
    dbg = {}
    if debug:
        for nm, shp in (("d_qt", [128, tok]), ("d_kt", [128, tok]),
                        ("d_v", [128, nh * 65]), ("d_ex", [128, min(1024, tok)]),
                        ("d_pv", [65, 512]), ("d_at", [128, tok])):
            dbg[nm] = nc.dram_tensor(nm, shp, F32, kind="ExternalOutput").ap()

    with tile.TileContext(nc) as tc:
        from contextlib import ExitStack
        with ExitStack() as ctx:
            # ---- persistent pools (live whole kernel) ----
            p_qk = ctx.enter_context(tc.tile_pool(name="p_qk", bufs=1))
            p_v = ctx.enter_context(tc.tile_pool(name="p_v", bufs=1))
            p_sm = ctx.enter_context(tc.tile_pool(name="p_sm", bufs=1))
            ps_wide = ctx.enter_context(
                tc.tile_pool(name="ps_wide", bufs=2, space="PSUM"))
            ps_bank = ctx.enter_context(
                tc.tile_pool(name="ps_bank", bufs=3, space="PSUM"))

            QT = [p_qk.tile([128, tok], mm_dt, tag=f"qt{i}", name=f"QT{i}")
                  for i in range(f_t)]
            KT = [p_qk.tile([128, tok], mm_dt, tag=f"kt{i}", name=f"KT{i}")
                  for i in range(f_t)]
            # V65: per token-tile, per head 64 V columns + a ones column
            V65 = [p_v.tile([128, nh * 65], mm_dt, tag=f"v{i}", name=f"V65_{i}")
                   for i in range(t_t)]
            ones64 = p_sm.tile([1, 64], F32, tag="ones64", name="ones64")
            nc.vector.memset(ones64[:, :], 1.0)
            onesf = p_sm.tile([128, 128], F32, tag="onesf", name="onesf")
            nc.vector.memset(onesf[:, :], 1.0)
            onestok = p_sm.tile([1, 128], mm_dt, tag="onestok", name="onestok")
            nc.vector.tensor_copy(onestok[:, :], onesf[0:1, :])
            bqs = p_sm.tile([128, f_t], F32, tag="bqs", name="bqs")
            bks = p_sm.tile([128, f_t], F32, tag="bks", name="bks")
            bvs = p_sm.tile([1, f], mm_dt, tag="bvs", name="bvs")
            nc.sync.dma_start(bqs[:, :], bq.rearrange("(a p) o -> p (a o)", p=128))
            nc.sync.dma_start(bks[:, :], bk.rearrange("(a p) o -> p (a o)", p=128))
            nc.sync.dma_start(bvs[:, :], bv[:, :])

            # ================= Phase 1: Q/K/V projections =================
            with ExitStack() as ph1:
                p_w = ph1.enter_context(tc.tile_pool(name="p_w", bufs=1))
                p_xt = ph1.enter_context(tc.tile_pool(name="p_xt", bufs=2))

                wq_s = [p_w.tile([128, f], mm_dt, tag=f"wq{i}", name=f"wq_s{i}")
                        for i in range(c_t)]
                wk_s = [p_w.tile([128, f], mm_dt, tag=f"wk{i}", name=f"wk_s{i}")
                        for i in range(c_t)]
                wv_s = [p_w.tile([128, f], mm_dt, tag=f"wv{i}", name=f"wv_s{i}")
                        for i in range(c_t)]
                for i in range(c_t):
                    nc.sync.dma_start(wq_s[i][:, :], wq[i * 128:(i + 1) * 128, :])
                    nc.sync.dma_start(wk_s[i][:, :], wk[i * 128:(i + 1) * 128, :])
                    nc.sync.dma_start(wv_s[i][:, :], wv[i * 128:(i + 1) * 128, :])

                for tch in range(tok // 512):
                    ts = slice(tch * 512, (tch + 1) * 512)
                    xts = []
                    for i in range(c_t):
                        xt_i = p_xt.tile([128, 512], mm_dt, tag=f"x{i}",
                                         name=f"xt{i}_{tch}")
                        nc.sync.dma_start(xt_i[:, :], xT[i * 128:(i + 1) * 128, ts])
                        xts.append(xt_i)
                    # QT / KT feature-major tiles
                    for (w_s, dst, bias) in ((wq_s, QT, bqs), (wk_s, KT, bks)):
                        for ft in range(f_t):
                            ps = ps_wide.tile([128, 512], F32, tag="sc",
                                              name=f"psqk{ft}_{tch}")
                            for i in range(c_t):
                                nc.tensor.matmul(
                                    ps[:, :],
                                    w_s[i][:, ft * 128:(ft + 1) * 128],
                                    xts[i][:, :],
                                    start=(i == 0), stop=(i == c_t - 1))
                            nc.vector.tensor_scalar_add(
                                dst[ft][:, ts], ps[:, :], bias[:, ft:ft + 1])
                    # V token-major + bias via ones-row matmul
                    for tt4 in range(4):
                        gt = tch * 4 + tt4  # global token tile
                        tsl = slice(tt4 * 128, (tt4 + 1) * 128)
                        psv = ps_bank.tile([128, f], F32, tag="pv",
                                           name=f"psv{gt}")
                        for i in range(c_t):
                            nc.tensor.matmul(
                                psv[:, :], xts[i][:, tsl], wv_s[i][:, :],
                                start=(i == 0), stop=False)
                        nc.tensor.matmul(psv[:, :], onestok[:, :],
                                         bvs[:, :], start=False, stop=True)
                        v_dst = V65[gt].rearrange("p (h e) -> p h e", e=65)
                        nc.vector.tensor_copy(v_dst[:, :, 64:65],
                                              onesf[:, 0:nh])
                        nc.vector.tensor_copy(
                            v_dst[:, :, 0:64],
                            psv.rearrange("p (h e) -> p h e", e=64)[:, :, :])

            if debug:
                nc.sync.dma_start(dbg["d_qt"][:, :], QT[0][:, :].bitcast(F32))
                nc.sync.dma_start(dbg["d_kt"][:, :], KT[0][:, :].bitcast(F32))
                nc.sync.dma_start(dbg["d_v"][:, :], V65[0][:, :].bitcast(F32))

            # ================= Phase 2: attention =================
            p_at = ctx.enter_context(tc.tile_pool(name="p_at", bufs=1))
            p_wo = ctx.enter_context(tc.tile_pool(name="p_wo", bufs=1))
            p_ex = ctx.enter_context(tc.tile_pool(name="p_ex", bufs=3))
            p_dn = ctx.enter_context(tc.tile_pool(name="p_dn", bufs=2))
            p_os = ctx.enter_context(tc.tile_pool(name="p_os", bufs=3))

            attnT = [p_at.tile([128, tok], mm_dt, tag=f"at{i}", name=f"attnT{i}")
                     for i in range(f_t)]
            wo_s = [p_wo.tile([128, cin], mm_dt, tag=f"wo{i}", name=f"wo_s{i}")
                    for i in range(f_t)]
            for i in range(f_t):
                nc.sync.dma_start(wo_s[i][:, :], wo[i * 128:(i + 1) * 128, :])

            for h in range(nh):
                ft, r0 = h // 2, (h % 2) * 64
                kq_rows = slice(r0, r0 + 64)
                vcol = slice(h * 65, h * 65 + 65)
                for ic in range(n_ic):
                    pvs = [ps_bank.tile([65, 512], F32, tag="pv",
                                        name=f"pv{h}_{ic}_{s}")
                           for s in range(nsub)]
                    for jt in range(t_t):
                        sc = ps_wide.tile([128, iw], F32, tag="sc",
                                          name=f"sc{h}_{ic}_{jt}")
                        for s in range(nsub):
                            i0 = ic * iw + s * 512
                            nc.tensor.matmul(
                                sc[:, s * 512:(s + 1) * 512],
                                KT[ft][kq_rows, jt * 128:(jt + 1) * 128],
                                QT[ft][kq_rows, i0:i0 + 512],
                                start=True, stop=True)
                        ex = p_ex.tile([128, iw], mm_dt, tag="ex",
                                       name=f"ex{h}_{ic}_{jt}")
                        nc.scalar.activation(ex[:, :], sc[:, :], AF.Exp,
                                             scale=SCALE)
                        if debug and h == 0 and ic == 0 and jt == 0:
                            nc.sync.dma_start(dbg["d_ex"][:, :], ex[:, :].bitcast(F32))
                        for s in range(nsub):
                            nc.tensor.matmul(
                                pvs[s][:, :], V65[jt][:, vcol],
                                ex[:, s * 512:(s + 1) * 512],
                                start=(jt == 0), stop=(jt == t_t - 1))
                    if debug and h == 0 and ic == 0:
                        dpv = p_dn.tile([65, 512], F32, tag="dpv", name="dpv")
                        nc.vector.tensor_copy(dpv[:, :], pvs[0][:, :])
                        nc.sync.dma_start(dbg["d_pv"][:, :], dpv[:, :])
                    for s in range(nsub):
                        i0 = ic * iw + s * 512
                        isl = slice(i0, i0 + 512)
                        nc.vector.tensor_copy(attnT[ft][kq_rows, isl],
                                              pvs[s][0:64, :])
                        dn = p_dn.tile([1, 512], F32, tag="dn",
                                       name=f"dn{h}_{ic}_{s}")
                        nc.vector.tensor_copy(dn[:, :], pvs[s][64:65, :])
                        dninv = p_dn.tile([1, 512], F32, tag="dninv",
                                          name=f"dninv{h}_{ic}_{s}")
                        nc.vector.reciprocal_approx_fast(
                            out=dninv[:, :], in_=dn[:, :])
                        rp = ps_bank.tile([64, 512], F32, tag="pv",
                                          name=f"rp{h}_{ic}_{s}")
                        nc.tensor.matmul(rp[:, :], ones64[:, :], dninv[:, :],
                                         start=True, stop=True)
                        nc.vector.tensor_mul(attnT[ft][kq_rows, isl],
                                             attnT[ft][kq_rows, isl], rp[:, :])

            if debug:
                nc.sync.dma_start(dbg["d_at"][:, :], attnT[0][:, :].bitcast(F32))

            # ================= Phase 3: output projection =================
            for tt in range(t_t):
                tsl = slice(tt * 128, (tt + 1) * 128)
                for oc in range(n_oc):
                    osl = slice(oc * ocw, (oc + 1) * ocw)
                    po = ps_wide.tile([128, ocw], F32, tag="sc",
                                      name=f"po{tt}_{oc}")
                    for i in range(f_t):
                        nc.tensor.matmul(po[:, :], attnT[i][:, tsl],
                                         wo_s[i][:, osl],
                                         start=(i == 0), stop=(i == f_t - 1))
                    ob = p_os.tile([128, ocw], F32, tag="os",
                                   name=f"ob{tt}_{oc}")
                    nc.vector.tensor_copy(ob[:, :], po[:, :])
                    nc.sync.dma_start(o_part[tsl, osl], ob[:, :])

    nc.finalize()
    return nc


_NC_CACHE = {}


def _get_nc(key=(N, C, NH, F32R)):
    if key not in _NC_CACHE:
        _NC_CACHE[key] = build_attention_kernel(*key)
    return _NC_CACHE[key]


def make_in_maps(x, Wq, bq, Wk, bk, Wv, bv, Wo):
    """Shard full inputs into 8 per-core input maps."""
    in_maps = []
    for c in range(NCORES):
        b, hg = divmod(c, 2)
        fs = slice(hg * F, (hg + 1) * F)
        in_maps.append({
            "xT": np.ascontiguousarray(x[b].T),
            "wq": np.ascontiguousarray(Wq[:, fs]),
            "wk": np.ascontiguousarray(Wk[:, fs]),
            "wv": np.ascontiguousarray(Wv[:, fs]),
            "bq": np.ascontiguousarray(bq[fs].reshape(F, 1)),
            "bk": np.ascontiguousarray(bk[fs].reshape(F, 1)),
            "bv": np.ascontiguousarray(bv[fs].reshape(1, F)),
            "wo": np.ascontiguousarray(Wo[fs, :]),
        })
    return in_maps


def kernel(x, Wq, bq, Wk, bk, Wv, bv, Wo, bo, **_unused):
    from concourse.bass_utils import run_bass_kernel_spmd

    arrs = [np.asarray(a, dtype=np.float32)
            for a in (x, Wq, bq, Wk, bk, Wv, bv, Wo, bo)]
    x, Wq, bq, Wk, bk, Wv, bv, Wo, bo = arrs

    nc = _get_nc()
    in_maps = make_in_maps(x, Wq, bq, Wk, bk, Wv, bv, Wo)
    res = run_bass_kernel_spmd(nc, in_maps, core_ids=list(range(NCORES)))

    out = np.empty((B, N, C), dtype=np.float32)
    for b in range(B):
        out[b] = res.results[2 * b]["o_part"] + res.results[2 * b + 1]["o_part"] + bo
    return out



# revision 5
# speedup vs baseline: 1.3129x; 1.3129x over previous
"""Multi-head self-attention Trainium2 kernel (8 NeuronCores).

Problem: x[4, 2048, 1024], H=16 heads, D=64. Sharding: core c handles
batch b = c // 2 and head-group hg = c % 2 (8 heads = 512 features).

Per-core math (F = 512 core-local features, T = 2048 tokens, C = 1024),
all matmul operands fp16 (PE runs fp16 at 1 cycle/row like bf16, with
~8x the mantissa), fp32 PSUM accumulation:

  QT = (Wq_s.T @ x_b.T) + bq_s          [F, T]   (feature-major)
  KT = same with Wk_s                    [F, T]
  V65 = [x_b @ Wv_s + bv_s | ones]       [T, 8*(64+1)] interleaved per head
  per (head pair, 512-query block):
    for each key tile kt (128 keys):
      scT[key, q] = KT_h.T @ QT_h        (two heads packed into the PE
                                          array via partition offsets 0/64)
      ex = exp(scT / 8)                  fp16 (no max subtraction: |s|<~2)
      pv[65, q] += V65_h[kt].T @ ex      (row 64 = softmax denominator)
    attnT_h[:, q] = pv[0:64] * (1/pv[64])  (recip on DVE, replicated
                                          across partitions via PE matmul)
  o_part = attnT.T @ Wo_s                [T, C]
Host: out[b] = o_part[2b] + o_part[2b+1] + bo.

Phase-2 normalization is software-pipelined: the PE-side part (the
replicate matmul + multiply) of block i is emitted in the middle of
block i+1's key loop so the PE never stalls on the DVE chain and the
HAM clock gate stays warm.
"""

import sys

import numpy as np

if "/opt/trn_rl_repo" not in sys.path:
    sys.path.insert(0, "/opt/trn_rl_repo")

import concourse.bass as bass
import concourse.mybir as mybir
import concourse.tile as tile
from concourse import bacc

F32 = mybir.dt.float32
F32R = mybir.dt.float32r
F16 = mybir.dt.float16
AF = mybir.ActivationFunctionType

# Full-problem constants
B, N, C, H, D = 4, 2048, 1024, 16, 64
NCORES = 8
NH = 8          # heads per core
F = NH * D      # 512 core-local features
SCALE = 1.0 / 8.0  # 1/sqrt(D)


def build_attention_kernel(tok=N, cin=C, nh=NH):
    """Build the per-core Bass program. Returns the finalized Bass object."""
    f = nh * D
    assert tok % 512 == 0 and cin % 128 == 0 and f % 128 == 0
    c_t = cin // 128       # contraction tiles for projections (8)
    f_t = f // 128         # feature tiles = head pairs (4)
    t_t = tok // 128       # token tiles (16)
    n_qb = tok // 512      # query blocks (4)
    n_oc = cin // 512      # output-proj column chunks (2)

    nc = bacc.Bacc("TRN2", target_bir_lowering=False, debug=False,
                   num_devices=NCORES)

    xT = nc.dram_tensor("xT", [cin, tok], F16, kind="ExternalInput").ap()
    wq = nc.dram_tensor("wq", [cin, f], F16, kind="ExternalInput").ap()
    wk = nc.dram_tensor("wk", [cin, f], F16, kind="ExternalInput").ap()
    wv = nc.dram_tensor("wv", [cin, f], F16, kind="ExternalInput").ap()
    bq = nc.dram_tensor("bq", [f, 1], F32, kind="ExternalInput").ap()
    bk = nc.dram_tensor("bk", [f, 1], F32, kind="ExternalInput").ap()
    bv = nc.dram_tensor("bv", [1, f], F16, kind="ExternalInput").ap()
    wo = nc.dram_tensor("wo", [f, cin], F16, kind="ExternalInput").ap()
    o_part = nc.dram_tensor("o_part", [tok, cin], F32,
                            kind="ExternalOutput").ap()

    with tile.TileContext(nc) as tc:
        from contextlib import ExitStack
        with ExitStack() as ctx:
            # ---- pools ----
            p_sm = ctx.enter_context(tc.tile_pool(name="p_sm", bufs=1))
            p_x = ctx.enter_context(tc.tile_pool(name="p_x", bufs=1))
            p_w = ctx.enter_context(tc.tile_pool(name="p_w", bufs=1))
            p_qk = ctx.enter_context(tc.tile_pool(name="p_qk", bufs=1))
            p_v = ctx.enter_context(tc.tile_pool(name="p_v", bufs=1))
            p_at = ctx.enter_context(tc.tile_pool(name="p_at", bufs=1))
            p_ex = ctx.enter_context(tc.tile_pool(name="p_ex", bufs=3))
            p_dn = ctx.enter_context(tc.tile_pool(name="p_dn", bufs=4))
            p_os = ctx.enter_context(tc.tile_pool(name="p_os", bufs=3))
            ps_sc = ctx.enter_context(
                tc.tile_pool(name="ps_sc", bufs=2, space="PSUM"))
            ps_pv = ctx.enter_context(
                tc.tile_pool(name="ps_pv", bufs=4, space="PSUM"))

            # ---- constants / biases ----
            ones64 = p_sm.tile([1, 64], F16, tag="ones64", name="ones64")
            nc.vector.memset(ones64[:, :], 1.0)
            onestok = p_sm.tile([1, 128], F16, tag="onestok", name="onestok")
            nc.vector.memset(onestok[:, :], 1.0)
            bqs = p_sm.tile([128, f_t], F32, tag="bqs", name="bqs")
            bks = p_sm.tile([128, f_t], F32, tag="bks", name="bks")
            bvs = p_sm.tile([1, f], F16, tag="bvs", name="bvs")
            nc.sync.dma_start(bqs[:, :], bq.rearrange("(a p) o -> p (a o)", p=128))
            nc.sync.dma_start(bks[:, :], bk.rearrange("(a p) o -> p (a o)", p=128))
            nc.sync.dma_start(bvs[:, :], bv[:, :])

            # ---- weight + x loads (DMA, overlapped with compute) ----
            wk_s = [p_w.tile([128, f], F16, tag=f"wk{i}", name=f"wk_s{i}")
                    for i in range(c_t)]
            wv_s = [p_w.tile([128, f], F16, tag=f"wv{i}", name=f"wv_s{i}")
                    for i in range(c_t)]
            wq_s = [p_w.tile([128, f], F16, tag=f"wq{i}", name=f"wq_s{i}")
                    for i in range(c_t)]
            xs = [p_x.tile([128, tok], F16, tag=f"x{i}", name=f"xs{i}")
                  for i in range(c_t)]
            wo_s = [p_w.tile([128, cin], F16, tag=f"wo{i}", name=f"wo_s{i}")
                    for i in range(f_t)]
            for i in range(c_t):
                nc.sync.dma_start(wk_s[i][:, :], wk[i * 128:(i + 1) * 128, :])
                nc.sync.dma_start(wv_s[i][:, :], wv[i * 128:(i + 1) * 128, :])
            for i in range(c_t):
                nc.sync.dma_start(xs[i][:, :], xT[i * 128:(i + 1) * 128, :])
            for i in range(c_t):
                nc.sync.dma_start(wq_s[i][:, :], wq[i * 128:(i + 1) * 128, :])
            for i in range(f_t):
                nc.sync.dma_start(wo_s[i][:, :], wo[i * 128:(i + 1) * 128, :])

            # ---- persistent activations ----
            KT = [p_qk.tile([128, tok], F16, tag=f"kt{i}", name=f"KT{i}")
                  for i in range(f_t)]
            QT = [p_qk.tile([128, tok], F16, tag=f"qt{i}", name=f"QT{i}")
                  for i in range(f_t)]
            V65 = [p_v.tile([128, nh * 65], F16, tag=f"v{i}", name=f"V65_{i}")
                   for i in range(t_t)]
            attnT = [p_at.tile([128, tok], F16, tag=f"at{i}", name=f"attnT{i}")
                     for i in range(f_t)]

            # ================= Phase 1: K, V, Q projections =================
            def qk_proj(w_s, dst, bias):
                for ft in range(f_t):
                    for tch in range(tok // 512):
                        ts = slice(tch * 512, (tch + 1) * 512)
                        ps = ps_sc.tile([128, 1024], F32, tag="sc",
                                        name=f"psqk{ft}_{tch}")
                        for i in range(c_t):
                            nc.tensor.matmul(
                                ps[:, 0:512],
                                w_s[i][:, ft * 128:(ft + 1) * 128],
                                xs[i][:, ts],
                                start=(i == 0), stop=(i == c_t - 1))
                        nc.vector.tensor_scalar_add(
                            dst[ft][:, ts], ps[:, 0:512], bias[:, ft:ft + 1])

            qk_proj(wk_s, KT, bks)

            # V: token-major, bias via ones-row matmul, interleaved ones col
            for gt in range(t_t):
                nc.vector.memset(V65[gt][:, :], 1.0)
            for gt in range(t_t):
                tsl = slice(gt * 128, (gt + 1) * 128)
                psv = ps_sc.tile([128, 1024], F32, tag="sc", name=f"psv{gt}")
                for i in range(c_t):
                    nc.tensor.matmul(
                        psv[:, 0:512], xs[i][:, tsl], wv_s[i][:, :],
                        start=(i == 0), stop=False)
                nc.tensor.matmul(psv[:, 0:512], onestok[:, :], bvs[:, :],
                                 start=False, stop=True)
                v_dst = V65[gt].rearrange("p (h e) -> p h e", e=65)
                nc.vector.tensor_copy(
                    v_dst[:, :, 0:64],
                    psv[:, 0:512].rearrange("p (h e) -> p h e", e=64)[:, :, :])

            qk_proj(wq_s, QT, bqs)

            # ================= Phase 2: attention =================
            pending = []  # deferred PE-side normalize of the previous block

            for pair in range(f_t):
                he, ho = 2 * pair, 2 * pair + 1
                for qb in range(n_qb):
                    qsl = slice(qb * 512, (qb + 1) * 512)
                    pv_e = ps_pv.tile([65, 512], F32, tag="pv",
                                      name=f"pv{pair}_{qb}e")
                    pv_o = ps_pv.tile([65, 512], F32, tag="pv",
                                      name=f"pv{pair}_{qb}o")
                    for kt in range(t_t):
                        if kt == 6 and pending:
                            for fn in pending:
                                fn()
                            pending = []
                        ksl = slice(kt * 128, (kt + 1) * 128)
                        sc = ps_sc.tile([128, 1024], F32, tag="sc",
                                        name=f"sc{pair}_{qb}_{kt}")
                        nc.tensor.matmul(sc[:, 0:512],
                                         KT[pair][0:64, ksl],
                                         QT[pair][0:64, qsl],
                                         start=True, stop=True)
                        nc.tensor.matmul(sc[:, 512:1024],
                                         KT[pair][64:128, ksl],
                                         QT[pair][64:128, qsl],
                                         start=True, stop=True)
                        ex = p_ex.tile([128, 1024], F16, tag="ex",
                                       name=f"ex{pair}_{qb}_{kt}")
                        nc.scalar.activation(ex[:, :], sc[:, :], AF.Exp,
                                             scale=SCALE)
                        nc.tensor.matmul(pv_e[:, :],
                                         V65[kt][:, he * 65:he * 65 + 65],
                                         ex[:, 0:512],
                                         start=(kt == 0), stop=(kt == t_t - 1))
                        nc.tensor.matmul(pv_o[:, :],
                                         V65[kt][:, ho * 65:ho * 65 + 65],
                                         ex[:, 512:1024],
                                         start=(kt == 0), stop=(kt == t_t - 1))

                    # part A (DVE only): unnormalized copy + reciprocal
                    for hh, pv in ((he, pv_e), (ho, pv_o)):
                        r0 = (hh % 2) * 64
                        attn_dst = attnT[pair][r0:r0 + 64, qsl]
                        nc.vector.tensor_copy(attn_dst, pv[0:64, :])
                        dnr = p_dn.tile([1, 512], F32, tag="dnr",
                                        name=f"dnr_{pair}_{qb}_{hh}")
                        nc.vector.tensor_copy(dnr[:, :], pv[64:65, :])
                        dninv32 = p_dn.tile([1, 512], F32, tag="dn32",
                                            name=f"dn32_{pair}_{qb}_{hh}")
                        nc.vector.reciprocal_approx_fast(
                            out=dninv32[:, :], in_=dnr[:, :])
                        dninv = p_dn.tile([1, 512], F16, tag="dn",
                                          name=f"dn{pair}_{qb}_{hh}")
                        nc.vector.tensor_copy(dninv[:, :], dninv32[:, :])

                        def part_b(attn_dst=attn_dst, dninv=dninv,
                                   pair=pair, qb=qb, hh=hh):
                            rp = ps_sc.tile([128, 1024], F32, tag="sc",
                                            name=f"rp{pair}_{qb}_{hh}")
                            nc.tensor.matmul(rp[0:64, 0:512], ones64[:, :],
                                             dninv[:, :],
                                             start=True, stop=True)
                            nc.vector.tensor_mul(attn_dst, attn_dst,
                                                 rp[0:64, 0:512])
                        pending.append(part_b)

            for fn in pending:
                fn()
            pending = []

            # ================= Phase 3: output projection =================
            for tt in range(t_t):
                tsl = slice(tt * 128, (tt + 1) * 128)
                for oc in range(n_oc):
                    osl = slice(oc * 512, (oc + 1) * 512)
                    po = ps_sc.tile([128, 1024], F32, tag="sc",
                                    name=f"po{tt}_{oc}")
                    for i in range(f_t):
                        nc.tensor.matmul(po[:, 0:512], attnT[i][:, tsl],
                                         wo_s[i][:, osl],
                                         start=(i == 0), stop=(i == f_t - 1))
                    ob = p_os.tile([128, 512], F32, tag="os",
                                   name=f"ob{tt}_{oc}")
                    nc.vector.tensor_copy(ob[:, :], po[:, 0:512])
                    nc.sync.dma_start(o_part[tsl, osl], ob[:, :])

    nc.finalize()
    return nc


_NC_CACHE = {}


def _get_nc(key=(N, C, NH)):
    if key not in _NC_CACHE:
        _NC_CACHE[key] = build_attention_kernel(*key)
    return _NC_CACHE[key]


def make_in_maps(x, Wq, bq, Wk, bk, Wv, bv, Wo):
    """Shard full inputs into 8 per-core input maps."""
    in_maps = []
    for c in range(NCORES):
        b, hg = divmod(c, 2)
        fs = slice(hg * F, (hg + 1) * F)
        in_maps.append({
            "xT": np.ascontiguousarray(x[b].T).astype(np.float16),
            "wq": np.ascontiguousarray(Wq[:, fs]).astype(np.float16),
            "wk": np.ascontiguousarray(Wk[:, fs]).astype(np.float16),
            "wv": np.ascontiguousarray(Wv[:, fs]).astype(np.float16),
            "bq": np.ascontiguousarray(bq[fs].reshape(F, 1)),
            "bk": np.ascontiguousarray(bk[fs].reshape(F, 1)),
            "bv": np.ascontiguousarray(bv[fs].reshape(1, F)).astype(np.float16),
            "wo": np.ascontiguousarray(Wo[fs, :]).astype(np.float16),
        })
    return in_maps


def kernel(x, Wq, bq, Wk, bk, Wv, bv, Wo, bo, **_unused):
    from concourse.bass_utils import run_bass_kernel_spmd

    arrs = [np.asarray(a, dtype=np.float32)
            for a in (x, Wq, bq, Wk, bk, Wv, bv, Wo, bo)]
    x, Wq, bq, Wk, bk, Wv, bv, Wo, bo = arrs

    nc = _get_nc()
    in_maps = make_in_maps(x, Wq, bq, Wk, bk, Wv, bv, Wo)
    res = run_bass_kernel_spmd(nc, in_maps, core_ids=list(range(NCORES)))

    out = np.empty((B, N, C), dtype=np.float32)
    for b in range(B):
        out[b] = res.results[2 * b]["o_part"] + res.results[2 * b + 1]["o_part"] + bo
    return out
